# revision 2
# baseline (speedup 1.0000x reference)
"""Trainium2 Bass kernel for nn_AttributeAttn — bf16-stream variant.

Same structure as kernel_v3 (replicated W streamed first, progressive u,
PE-built score tiles, finish chains spliced into the stream), but the
three big read-once tensors (hidden 16MB, W 6MB, enc 0.5MB per core) are
packed to bf16 on the host, halving HBM traffic to ~12.3MB per core.
All accumulation stays f32 in PSUM; b@v bias stays f32; output is f32.
Expected output rel err ~1e-3 against the f32 reference (gate is 2e-2).

Distribution: data-parallel over B (4 batches per core, 8 cores).
"""
import sys
import types

import numpy as np
import ml_dtypes

if "antenv.axon_hooks" not in sys.modules:
    _hooks_mod = types.ModuleType("antenv.axon_hooks")
    try:
        from trn_agent_boot.trn_boot import _ntff_profile_via_ctypes
        _ntff_hook = _ntff_profile_via_ctypes("/opt/axon/libaxon_pjrt.so")
    except Exception:
        _ntff_hook = None
    _hooks_mod.get_axon_ntff_profile_hook = lambda: _ntff_hook
    _hooks_mod.set_axon_ntff_profile_hook = lambda h: None
    sys.modules["antenv.axon_hooks"] = _hooks_mod

import concourse.bacc as bacc
import concourse.tile as tile
from concourse import mybir
from concourse.bass_utils import run_bass_kernel_spmd

f32 = mybir.dt.float32
f32r = mybir.dt.float32r
bf16 = mybir.dt.bfloat16
np_bf16 = ml_dtypes.bfloat16
AF = mybir.ActivationFunctionType
X = mybir.AxisListType.X
ADD = mybir.AluOpType.add

N, B, H = 1024, 32, 1024
C, K = 64, 512
NCORES = 8
BPC = B // NCORES            # 4 batches per core
NB = N * BPC                 # 4096 free elements of the hv contraction
HC = H // 128                # 8 h-chunks
KC = K // 128                # 4 k-chunks
JC = (H + K) // 128          # 12 u columns
NBLK = N // 128              # 8 n-blocks per core
NPAIR = NBLK // 2            # contraction runs in 4 pairs of n-blocks
FW = BPC * C                 # 256 free (bb, c) elements per n-block

TRACE = False
TRACE_KW = {}
LAST_RESULT = None

_cached = None


def _build():
    nc = bacc.Bacc(None, target_bir_lowering=False)
    hid_d = nc.dram_tensor("hid", [H, NB], bf16, kind="ExternalInput")
    enc_d = nc.dram_tensor("enc", [K, FW], bf16, kind="ExternalInput")
    w_d = nc.dram_tensor("w", [H, H + K], bf16, kind="ExternalInput")
    vb_d = nc.dram_tensor("vb", [128, 2 * HC], f32, kind="ExternalInput")
    vb16_d = nc.dram_tensor("vb16", [128, 2 * HC], bf16, kind="ExternalInput")
    out_d = nc.dram_tensor("out", [N, FW], f32, kind="ExternalOutput")

    with tile.TileContext(nc) as tc:
        with (
            tc.tile_pool(name="consts", bufs=1) as consts,
            tc.tile_pool(name="wpool", bufs=2) as wpool,
            tc.tile_pool(name="hpool", bufs=1) as hpool,
            tc.tile_pool(name="work", bufs=3) as work,
        ):
            rings = [nc.sync, nc.scalar]
            # --- constants + W stream first: u must be ready before the
            # first hidden contraction, and W-first costs nothing (the
            # kernel end is gated by total bytes, not by internal order).
            vb_sb = consts.tile([128, 2 * HC], f32, tag="vb")
            nc.sync.dma_start(out=vb_sb, in_=vb_d[:, :])
            vb16_sb = consts.tile([128, 2 * HC], bf16, tag="vb16")
            nc.sync.dma_start(out=vb16_sb, in_=vb16_d[:, :])
            w_tiles = []
            for ic in range(HC):
                # unique slot per chunk (24KB/partition in bf16): a reused
                # slot would make chunk ic+N's DMA wait on chunk ic's u
                # matmuls, and every hidden DMA queued FIFO behind it on
                # the same ring would stall with it
                w_sb = wpool.tile([128, H + K], bf16, tag="w", bufs=HC,
                                  name=f"w_{ic}")
                rings[ic % 2].dma_start(
                    out=w_sb,
                    in_=w_d[ic * 128:(ic + 1) * 128, :])
                w_tiles.append(w_sb)
            enc_sb = consts.tile([128, KC, FW], bf16, tag="enc")
            nc.scalar.dma_start(
                out=enc_sb,
                in_=enc_d.rearrange("(kc p) f -> p kc f", p=128))
            ones_sb = consts.tile([1, 128], f32, tag="ones")
            nc.vector.memset(ones_sb, 1.0)
            ones_r = consts.tile([1, 128], f32r, tag="ones_r")
            nc.vector.tensor_copy(ones_r, ones_sb)
            # bf16 ones: rank-1 transposes in bf16 are single-pass on the
            # PE; fp32 rank-1s lower to two LOW_HIGH passes (2x the time)
            ones16 = consts.tile([1, 128], bf16, tag="ones16")
            nc.vector.tensor_copy(ones16, ones_sb)

            # --- hidden stream: 32 [128,1024] tiles (one per h-chunk x
            # n-block-pair), pair-major so block pairs complete in order.
            # Unique SBUF slot per tile (16MB total): zero backpressure.
            hid_sb = {}
            for p in range(NPAIR):
                for hc in range(HC):
                    t = hpool.tile([128, 1024], bf16, tag=f"hid{hc}",
                                   bufs=NPAIR, name=f"hid_{hc}_{p}")
                    rings[hc % 2].dma_start(
                        out=t,
                        in_=hid_d[
                            hc * 128:(hc + 1) * 128, p * 1024:(p + 1) * 1024])
                    hid_sb[hc, p] = t

            # --- u row = v.T @ W, consuming W chunks as they arrive ---
            with tc.tile_pool(name="ps_setup", bufs=1, space="PSUM") as pset:
                u_ps = pset.tile([1, 3, 512], f32, tag="u")
                for ic in range(HC):
                    for jb in range(3):
                        nc.tensor.matmul(
                            u_ps[:, jb, :],
                            vb16_sb[:, ic:ic + 1],
                            w_tiles[ic][:, jb * 512:(jb + 1) * 512],
                            start=(ic == 0), stop=(ic == HC - 1))
                u_row = consts.tile([1, JC * 128], bf16, tag="urow")
                nc.vector.tensor_copy(
                    u_row.rearrange("p (a b) -> p a b", a=3), u_ps)

                # bias = b @ v
                bias_ps = pset.tile([1, 1], f32, tag="bias")
                for ic in range(HC):
                    nc.tensor.matmul(
                        bias_ps, vb_sb[:, ic:ic + 1],
                        vb_sb[:, HC + ic:HC + ic + 1],
                        start=(ic == 0), stop=(ic == HC - 1))
                bias_sb = consts.tile([1, 1], f32, tag="bias_sb")
                nc.vector.tensor_copy(bias_sb, bias_ps)

                # u columns (128, 12) via rank-1 transposes (bf16, 1-pass)
                uc_ps = pset.tile([128, JC], f32, tag="uc")
                for jc in range(JC):
                    nc.tensor.matmul(
                        uc_ps[:, jc:jc + 1],
                        u_row[0:1, jc * 128:(jc + 1) * 128],
                        ones16[:, 0:1], start=True, stop=True)
                ucols = consts.tile([128, JC], bf16, tag="ucols")
                nc.vector.tensor_copy(ucols, uc_ps)

                # ev row (1, 256) then evb = ev + bias
                ev_ps = pset.tile([1, FW], f32, tag="ev")
                for kc in range(KC):
                    nc.tensor.matmul(
                        ev_ps, ucols[:, HC + kc:HC + kc + 1],
                        enc_sb[:, kc, :], start=(kc == 0), stop=(kc == KC - 1))
                evb_row = consts.tile([1, FW], f32r, tag="evb")
                nc.vector.tensor_scalar_add(evb_row, ev_ps, bias_sb[:, 0:1])

            # --- main: hv contraction per n-block pair; score + softmax
            # finish chains spliced between the next pair's matmuls. ---
            with tc.tile_pool(name="ps_main", bufs=1, space="PSUM") as pp:
                rows = {}

                def contract_pair(p, pending):
                    accs = {k: pp.tile([1, 512], f32, tag="acc", bufs=4,
                                       name=f"acc_{k}")
                            for k in (2 * p, 2 * p + 1)}
                    for hc in range(HC):
                        for k in (2 * p, 2 * p + 1):
                            nc.tensor.matmul(
                                accs[k], ucols[:, hc:hc + 1],
                                hid_sb[hc, p][:, (k % 2) * 512:
                                              (k % 2) * 512 + 512],
                                start=(hc == 0), stop=(hc == HC - 1))
                        # previous pair's finish work slots between chunk
                        # batches instead of queueing behind this pair's
                        # last matmul
                        if hc in (1, 5) and pending:
                            finish(pending.pop(0))
                    for k in (2 * p, 2 * p + 1):
                        # bf16 so the finish rank-1s are single-pass
                        row = work.tile([1, 512], bf16, tag="row",
                                        name=f"row_{k}", bufs=NBLK)
                        nc.vector.tensor_copy(row, accs[k])
                        rows[k] = row

                def finish(k, tail=False):
                    # score built directly in PSUM by TensorE: no hv
                    # transpose or partition-broadcast pass needed.
                    rowv = rows[k].rearrange("p (n bb) -> p bb n", bb=BPC)
                    sc_ps = pp.tile([128, FW], f32, tag="score", bufs=3,
                                    name=f"score_{k}")
                    nc.tensor.matmul(
                        sc_ps, ones_r, evb_row, start=True, stop=False)
                    for bb in range(BPC):
                        nc.tensor.matmul(
                            sc_ps[:, bb * C:(bb + 1) * C],
                            rowv[0:1, bb, :], ones16[:, 0:C],
                            start=False, stop=(bb == BPC - 1),
                            skip_group_check=True)
                    sc = work.tile([128, FW], f32, tag="sc")
                    nc.scalar.activation(out=sc, in_=sc_ps, func=AF.Tanh)
                    nc.scalar.activation(out=sc, in_=sc, func=AF.Exp)
                    den = work.tile([128, BPC], f32, tag="den")
                    nc.vector.tensor_reduce(
                        den, sc.rearrange("p (bb c) -> p bb c", c=C),
                        axis=X, op=ADD)
                    nc.vector.reciprocal(den, den)
                    o_sb = work.tile([128, FW], f32, tag="o")
                    for bb in range(BPC):
                        nc.vector.tensor_scalar_mul(
                            o_sb[:, bb * C:(bb + 1) * C],
                            sc[:, bb * C:(bb + 1) * C],
                            den[:, bb:bb + 1])
                    if tail:
                        # HWDGE rings are idle at the tail and have the
                        # lowest store latency
                        rings[k % 2].dma_start(
                            out=out_d[k * 128:(k + 1) * 128, :], in_=o_sb)
                    else:
                        nc.gpsimd.dma_start(
                            out=out_d[k * 128:(k + 1) * 128, :], in_=o_sb)

                pending = []
                for p in range(NPAIR):
                    contract_pair(p, pending)
                    pending.extend((2 * p, 2 * p + 1))
                for k in pending:
                    finish(k, tail=True)
    nc.compile()
    return nc


def kernel(**inputs):
    global _cached, LAST_RESULT
    hidden = np.ascontiguousarray(inputs["hidden"], dtype=np.float32)
    enc = np.ascontiguousarray(inputs["encoder_outputs"], dtype=np.float32)
    W = np.ascontiguousarray(inputs["W"], dtype=np.float32)
    b = np.ascontiguousarray(inputs["b"], dtype=np.float32)
    v = np.ascontiguousarray(inputs["v"], dtype=np.float32)

    if _cached is None:
        _cached = _build()
    nc = _cached

    # vb: column ic holds v[ic*128:(ic+1)*128]; column HC+ic holds b chunk.
    vb = np.concatenate(
        [v.reshape(HC, 128).T, b.reshape(HC, 128).T], axis=1)
    vb = np.ascontiguousarray(vb, dtype=np.float32)
    vb16 = np.ascontiguousarray(vb.astype(np_bf16))
    W16 = np.ascontiguousarray(W.astype(np_bf16))

    in_maps = []
    for j in range(NCORES):
        bsl = slice(j * BPC, (j + 1) * BPC)
        # (H, N, bpc) -> (H, N*bpc); free index = n*BPC + bb
        hid_t = np.ascontiguousarray(
            np.transpose(hidden[:, bsl, :], (2, 0, 1)).reshape(H, NB)
            .astype(np_bf16))
        # (K, bpc, C) -> (K, bpc*C); free index = bb*C + c
        enc_t = np.ascontiguousarray(
            np.transpose(enc[:, bsl, :], (2, 1, 0)).reshape(K, FW)
            .astype(np_bf16))
        in_maps.append({"hid": hid_t, "enc": enc_t, "w": W16,
                        "vb": vb, "vb16": vb16})

    res = run_bass_kernel_spmd(
        nc, in_maps, core_ids=list(range(NCORES)), trace=TRACE, **TRACE_KW)
    LAST_RESULT = res

    out = np.empty((B, N, C), dtype=np.float32)
    for j in range(NCORES):
        o = res.results[j]["out"].reshape(N, BPC, C)
        out[j * BPC:(j + 1) * BPC] = o.transpose(1, 0, 2)
    return out
